# revision 35
# baseline (speedup 1.0000x reference)
"""Trainium2 Bass kernel for nn_BasicTransformerLayer (dense transformer layer).

Strategy:
- Data-parallel over batch: B=8, one batch element per NeuronCore, no collectives.
- All activations kept TRANSPOSED [features, tokens]; host pre-transposes.
- fp16 storage for weights + activations (halves DMA, enables DVE 2x/4x fast
  modes); all matmul accumulation in fp32 PSUM; LN statistics in fp32.
- Softmax without max-subtraction: p = exp(scores) * exp(bias) with exp(bias)
  precomputed host-side (fp16). The kv-sum (softmax denominator) is folded
  into the P@V matmul: each head's V block carries 64 extra ones-columns, so
  the PV psum rows 64..127 hold the denominator replicated across partitions
  (broadcast-free normalize).
- Score tiles are paired [128, 2*512] across 2 PSUM banks so one Exp
  activation and one eb-multiply cover 2 kv tiles.
- FFN w1 is cached whole in SBUF (fp16, prefetched early on gpsimd queue);
  w2 is streamed fp16. FFN runs without DMA stalls.
"""
import sys

sys.path.insert(0, '/opt/trn_rl_repo')

import numpy as np

E, C, H, D, FF = 768, 512, 12, 64, 3072
B, S, L = 8, 1024, 256
EPS = 1e-5
NCORES = 8
QCH = 512                  # q-chunk (matmul moving free dim)
NQ = S // QCH              # 2
JE = E // 128              # 6 feature tiles
JC = C // 128              # 4 cross-feature tiles
JF = FF // 128             # 24 ffn tiles
KVS = S // 128             # 8 self kv tiles
KVC = L // 128             # 2 cross kv tiles
VB = 128                   # per-head V block in V tiles: 64 data + 64 ones

_BUILT = {}
TRACE = False
LAST = {}
PHASES = []


def _build(flags):
    import concourse.bacc as bacc
    import concourse.mybir as mybir
    import concourse.tile as tile
    from concourse.tile import add_dep_helper

    F32 = mybir.dt.float32
    F16 = mybir.dt.float16
    AF = mybir.ActivationFunctionType
    OP = mybir.AluOpType

    nc = bacc.Bacc("TRN2", target_bir_lowering=False, debug=False,
                   enable_asserts=True, num_devices=NCORES)

    def din(name, shape, dt=F16):
        return nc.dram_tensor(name, shape, dt, kind="ExternalInput").ap()

    xT_d = din("xT", [E, S])
    ctxT_d = din("ctxT", [C, L])
    w_d = {
        'wq_c': din("wq_c", [E, E]), 'wk_c': din("wk_c", [C, E]),
        'wv_c': din("wv_c", [C, E]), 'wo_c': din("wo_c", [E, E]),
        'wq_s': din("wq_s", [E, E]), 'wk_s': din("wk_s", [E, E]),
        'wv_s': din("wv_s", [E, E]), 'wo_s': din("wo_s", [E, E]),
        'w1': din("w1", [E, FF]), 'w2': din("w2", [FF, E]),
    }
    expb_c_d = din("expb_c", [H, L, S])
    expb_s_d = din("expb_s", [H, S, S])
    # vecs: [128, NV] -- per-partition 128-chunks of all small vectors
    VIDX = {}
    _off = 0
    for nm, ln in [('cn_g', JE), ('cn_b', JE), ('sn_g', JE), ('sn_b', JE),
                   ('fn_g', JE), ('fn_b', JE), ('bq_c', JE), ('bk_c', JE),
                   ('bo_c', JE), ('bq_s', JE), ('bk_s', JE), ('bo_s', JE),
                   ('b1', JF), ('b2', JE)]:
        VIDX[nm] = _off
        _off += ln
    NV = _off
    vecs_d = din("vecs", [128, NV], F32)
    yT_d = nc.dram_tensor("yT", [E, S], F32, kind="ExternalOutput").ap()

    with tile.TileContext(nc) as tc:
        with tc.tile_pool(name="const", bufs=1) as cpool, \
             tc.tile_pool(name="acts", bufs=1) as acts, \
             tc.tile_pool(name="wst", bufs=8) as wst, \
             tc.tile_pool(name="tr", bufs=2) as tr, \
             tc.tile_pool(name="pe", bufs=6) as pepool, \
             tc.tile_pool(name="eb", bufs=4) as ebpool, \
             tc.tile_pool(name="ps", bufs=1, space="PSUM") as ps:

            def T(pool, shape, dtype, tag):
                return pool.tile(shape, dtype, tag=tag, name=tag)

            ones = T(cpool, [128, 128], F16, "ones")
            epsc = T(cpool, [128, 1], F32, "epsc")
            nc.vector.memset(epsc[:], EPS)
            nc.vector.memset(ones[:], 1.0)
            vecs = T(cpool, [128, NV], F32, "vecs")
            nc.sync.dma_start(vecs[:], vecs_d[:])
            # whole w1 cached in SBUF: col j*FF + of*128 + c = w1[j*128+p, of*128+c]
            w1s = T(cpool, [128, JE * FF], F16, "w1s")

            def vap(nm, j):
                return vecs[:, VIDX[nm] + j:VIDX[nm] + j + 1]

            # persistent activation tiles (tags reused across phases)
            rA = [T(acts, [128, S], F16, f"rA{j}") for j in range(JE)]  # xT then h2T
            rB = [T(acts, [128, S], F16, f"rB{j}") for j in range(JE)]  # h1T
            lnT = [T(acts, [128, S], F16, f"ln{j}") for j in range(JE)]
            # Per-head K/Q tiles: head data in partitions 0-63, zeros in
            # 64-127, so score matmuls are uniform 128-contraction (the PE
            # runs much slower when 64-row and 128-row weight tiles alternate)
            KZ = [T(acts, [128, S], F16, f"KZ{h}") for h in range(H)]
            QZ = [T(acts, [128, QCH], F16, f"QZ{h}") for h in range(H)]
            # V tiles: per head 64 data cols + 64 ones cols
            V = [T(acts, [128, H * VB], F16, f"V{t}") for t in range(KVS)]
            AT = [T(acts, [128, QCH], F16, f"AT{j}") for j in range(JE)]
            ctxT = [T(acts, [128, L], F16, f"cx{j}") for j in range(JC)]
            # V block layout per head: [data(64) | ones(64)] -> PV psum rows
            # 0-63 hold the PV, 64-127 the denominator. The ones/zeros init
            # memsets are emitted inside _ln1 (after the prologue copies) so
            # they don't block the Vector queue at startup; they only touch
            # regions the projection copies never write.
            for j in range(JC):
                nc.sync.dma_start(ctxT[j][:], ctxT_d[j * 128:(j + 1) * 128, :])

            # PSUM layout: pair tags P0/P1/P2 = [128,1024] (2 banks each),
            # singles B5/B6. prologue: V-chains P0/P1, wform rot B5/B6,
            # ln s1/s2 = halves of P0. inner loop: paired score tiles rotate
            # P0-P2, pv B5/B6. FFN: y accumulators = P0/P1/P2 halves,
            # f1 B5/B6.
            def psum_tile(tag, n=QCH):
                return ps.tile([128, n], F32, tag=tag, name=tag)

            _rot = {'i': 0}

            def rot_sm():
                t = ("B5", "B6")[_rot['i'] % 2]
                _rot['i'] += 1
                return t

            # ---------------- layer norm (transposed layout) ----------------
            def ln_phase(src, dst, gname, bname, affine, only_qc=None):
                inv = 1.0 / float(E)
                for qc in range(NQ):
                    if only_qc is not None and qc != only_qc:
                        continue
                    qs = slice(qc * QCH, (qc + 1) * QCH)
                    sqs = []
                    for j in range(JE):
                        sq = tr.tile([128, QCH], F16, tag="sq", name="sq", bufs=6)
                        nc.vector.tensor_tensor(sq[:], src[j][:, qs],
                                                src[j][:, qs], op=OP.mult)
                        sqs.append(sq)
                    lnp = psum_tile("P0", 2 * QCH)
                    s1 = lnp[:, 0:QCH]
                    s2 = lnp[:, QCH:2 * QCH]
                    for j in range(JE):
                        nc.tensor.matmul(s1, ones[:, 0:128], src[j][:, qs],
                                         start=(j == 0), stop=(j == JE - 1))
                    for j in range(JE):
                        nc.tensor.matmul(s2, ones[:, 0:128], sqs[j][:],
                                         start=(j == 0), stop=(j == JE - 1))
                    t1 = tr.tile([128, QCH], F32, tag="t1m", name="t1m", bufs=1)
                    nc.scalar.activation(t1[:], s1, AF.Square, scale=inv)
                    var = tr.tile([128, QCH], F32, tag="var", name="var", bufs=1)
                    nc.vector.scalar_tensor_tensor(var[:], s2, inv, t1[:],
                                                   op0=OP.mult, op1=OP.subtract)
                    # rstd = (var+eps)^-1/2 via ln+exp: both live in the same
                    # ACT table as the attention Exp, so no table swaps
                    lnv = tr.tile([128, QCH], F32, tag="lnv", name="lnv", bufs=1)
                    nc.scalar.activation(lnv[:], var[:], AF.Ln, bias=epsc[:, 0:1])
                    rstd = tr.tile([128, QCH], F16, tag="rstd", name="rstd", bufs=1)
                    nc.scalar.activation(rstd[:], lnv[:], AF.Exp, scale=-0.5)
                    m1r = tr.tile([128, QCH], F16, tag="m1r", name="m1r", bufs=1)
                    nc.vector.scalar_tensor_tensor(m1r[:], s1, inv, rstd[:],
                                                   op0=OP.mult, op1=OP.mult)
                    for j in range(JE):
                        tmp = tr.tile([128, QCH], F16, tag="lntmp", name="lntmp",
                                      bufs=1)
                        nc.vector.tensor_tensor(tmp[:], src[j][:, qs], rstd[:],
                                                op=OP.mult)
                        if affine:
                            tmp2 = tr.tile([128, QCH], F16, tag="lntmp2",
                                           name="lntmp2")
                            nc.vector.tensor_tensor(tmp2[:], tmp[:], m1r[:],
                                                    op=OP.subtract)
                            nc.vector.tensor_scalar(dst[j][:, qs], tmp2[:],
                                                    vap(gname, j), vap(bname, j),
                                                    op0=OP.mult, op1=OP.add)
                        else:
                            nc.vector.tensor_tensor(dst[j][:, qs], tmp[:], m1r[:],
                                                    op=OP.subtract)

            # ---- projection of one 128-feature block, split into 2 per-head
            # tiles (data at partitions 0-63; rows 64-127 stay zero) ----
            def proj_heads(wd, jin, src_getter, dst, of, ks, n, bflag, bname):
                pt = psum_tile(rot_sm(), n)
                wt = wst.tile([128, JE * 128], F16, tag="wg", name="wg", bufs=3)
                nc.sync.dma_start(
                    wt[:, 0:jin * 128].rearrange("p (j c) -> p j c", j=jin),
                    wd[0:jin * 128, of * 128:(of + 1) * 128]
                    .rearrange("(j p) c -> p j c", p=128))
                for j in range(jin):
                    nc.tensor.matmul(pt[:, 0:n], wt[:, j * 128:(j + 1) * 128],
                                     src_getter(j),
                                     start=(j == 0), stop=(j == jin - 1))
                src = pt
                if bflag:
                    tmpb = tr.tile([128, QCH], F32, tag="projb", name="projb",
                                   bufs=2)
                    nc.vector.tensor_scalar(tmpb[:, 0:n], pt[:, 0:n],
                                            vap(bname, of), None, op0=OP.add)
                    src = tmpb
                nc.vector.tensor_copy(dst[2 * of][0:D, ks:ks + n],
                                      src[0:D, 0:n])
                nc.scalar.copy(dst[2 * of + 1][0:D, ks:ks + n],
                               src[D:128, 0:n])

            # ---------------- attention (shared cross/self) ----------------
            def attention(prefix, lnt, kv_src, expb_d, res_in,
                          res_out, wq, wk, wv, wo, jin_kv, kv_len,
                          pre_loop=None, post_qc=None):
                nkv = kv_len // 128

                def emit_k(of):
                    for ks in range(0, kv_len, QCH):
                        n = min(QCH, kv_len - ks)
                        proj_heads(wk, jin_kv,
                                   lambda j: kv_src[j][:, ks:ks + n],
                                   KZ, of, ks, n,
                                   flags[f'bk_{prefix}'], f'bk_{prefix}')

                def emit_vgroup(os_, tg):
                    n = min(QCH, E - os_)
                    tcnt = min(4, nkv - tg)
                    vbig = [psum_tile("P0", 2 * QCH), psum_tile("P1", 2 * QCH)]
                    vps = [vbig[i // 2][:, (i % 2) * QCH:(i % 2) * QCH + n]
                           for i in range(tcnt)]
                    for j in range(jin_kv):
                        wt = wst.tile([128, QCH], F16, tag="wv", name="wv", bufs=2)
                        nc.sync.dma_start(wt[:, 0:n],
                                          wv[j * 128:(j + 1) * 128, os_:os_ + n])
                        for i in range(tcnt):
                            nc.tensor.matmul(
                                vps[i],
                                kv_src[j][:, (tg + i) * 128:(tg + i + 1) * 128],
                                wt[:, 0:n], start=(j == 0), stop=(j == jin_kv - 1))
                    h0, nh = os_ // D, n // D
                    for i in range(tcnt):
                        dst = V[tg + i][:].rearrange(
                            "p (g c) -> p g c", c=VB)[:, h0:h0 + nh, 0:D]
                        src = vps[i].rearrange("p (g c) -> p g c", c=D)
                        nc.scalar.copy(dst, src)

                vgroups = [(os_, tg) for os_ in range(0, E, QCH)
                           for tg in range(0, nkv, 4)]
                for i in range(max(JE, len(vgroups))):
                    if i < JE:
                        emit_k(i)
                    if i < len(vgroups):
                        emit_vgroup(*vgroups[i])

                if pre_loop is not None:
                    pre_loop()

                for qc in range(NQ):
                    PHASES.append((f'{prefix}:qc{qc}',
                                   int(__import__('re').findall(
                                       r'\d+', nc.get_next_instruction_name())[0])))
                    qs = slice(qc * QCH, (qc + 1) * QCH)
                    # Q^T for this q-chunk (scale folded into wq on host)
                    for of in range(JE):
                        proj_heads(wq, JE, lambda j: lnt[j][:, qs],
                                   QZ, of, 0, QCH,
                                   flags[f'bq_{prefix}'], f'bq_{prefix}')
                    nkt = kv_len // 128
                    npairs = nkt // 2
                    seq = [(h, pr) for h in range(H) for pr in range(npairs)]
                    SC_TAGS = ["P0", "P1", "P2"]
                    _sr = {'i': 0}
                    state = {}

                    def load_eb(h):
                        ebts = []
                        for b0 in range(0, nkt, 2):
                            ebt = ebpool.tile([128, 2 * QCH], F16, tag="eb",
                                              name="eb", bufs=6)
                            nc.gpsimd.dma_start(
                                ebt[:].rearrange("p (t c) -> p t c", t=2),
                                expb_d[h, b0 * 128:(b0 + 2) * 128, qs]
                                .rearrange("(t p) c -> p t c", p=128))
                            ebts.append(ebt)
                        state.setdefault(h, {'tiles': []})['ebts'] = ebts

                    load_eb(0)

                    _pe_chain = {'prev': None}

                    def chain(bi):
                        if _pe_chain['prev'] is not None:
                            add_dep_helper(bi.ins, _pe_chain['prev'].ins,
                                           sync=False, reason="pe-order")
                        _pe_chain['prev'] = bi

                    def s_stage(i):
                        h, pr = seq[i]
                        st = state.setdefault(h, {'tiles': []})
                        if pr == 0 and h + 1 < H:
                            load_eb(h + 1)
                        sc = psum_tile(SC_TAGS[_sr['i'] % 3], 2 * QCH)
                        _sr['i'] += 1
                        for t in range(2):
                            kvt = 2 * pr + t
                            chain(nc.tensor.matmul(
                                sc[:, t * QCH:(t + 1) * QCH],
                                KZ[h][:, kvt * 128:(kvt + 1) * 128],
                                QZ[h][:], start=True, stop=True))
                        pe = pepool.tile([128, 2 * QCH], F16, tag="pe",
                                         name="pe", bufs=6)
                        nc.scalar.activation(pe[:], sc[:], AF.Exp)
                        nc.vector.tensor_tensor(pe[:], pe[:],
                                                st['ebts'][pr][:], op=OP.mult)
                        st['tiles'].append(pe)

                    def a_stage(i):
                        h, pr = seq[i]
                        st = state[h]
                        th, ph = (h * D) // 128, (h * D) % 128
                        if pr == 0:
                            st['pv'] = psum_tile(("B5", "B6")[h % 2])
                        pe = st['tiles'][pr]
                        for t in range(2):
                            kvt = 2 * pr + t
                            chain(nc.tensor.matmul(
                                st['pv'][:], V[kvt][:, h * VB:(h + 1) * VB],
                                pe[:, t * QCH:(t + 1) * QCH],
                                start=(kvt == 0), stop=(kvt == nkt - 1)))
                        if pr == npairs - 1:
                            # psum rows 0-63 = PV, 64-127 = denominator
                            # (replicated). DVE cannot partition-shift, ACT
                            # can: copy denominator down via Scalar, then
                            # aligned DVE recip + normalize.
                            den = tr.tile([128, QCH], F32, tag="den", name="den",
                                          bufs=2)
                            nc.scalar.copy(den[0:D, :], st['pv'][D:2 * D, :])
                            rec = tr.tile([128, QCH], F32, tag="rec", name="rec",
                                          bufs=2)
                            nc.vector.reciprocal_approx_fast(
                                rec[0:D, :], den[0:D, :])
                            nc.vector.tensor_tensor(AT[th][ph:ph + D, :],
                                                    st['pv'][0:D, :],
                                                    rec[0:D, :], op=OP.mult)
                            del state[h]

                    LOOK = 3
                    for i in range(len(seq) + LOOK):
                        if i < len(seq):
                            s_stage(i)
                        if i >= LOOK:
                            a_stage(i - LOOK)
                    # out-projection + residual
                    for of in range(JE):
                        pt = psum_tile(rot_sm())
                        wt = wst.tile([128, JE * 128], F16, tag="wg", name="wg",
                                      bufs=3)
                        nc.sync.dma_start(
                            wt[:].rearrange("p (j c) -> p j c", j=JE),
                            wo[0:E, of * 128:(of + 1) * 128]
                            .rearrange("(j p) c -> p j c", p=128))
                        for j in range(JE):
                            nc.tensor.matmul(pt[:], wt[:, j * 128:(j + 1) * 128],
                                             AT[j][:],
                                             start=(j == 0), stop=(j == JE - 1))
                        if flags[f'bo_{prefix}']:
                            nc.vector.scalar_tensor_tensor(
                                res_out[of][:, qs], pt[:], vap(f'bo_{prefix}', of),
                                res_in[of][:, qs], op0=OP.add, op1=OP.add)
                        else:
                            nc.vector.tensor_tensor(res_out[of][:, qs], pt[:],
                                                    res_in[of][:, qs], op=OP.add)
                    if post_qc is not None:
                        post_qc(qc)

            # ================= the layer =================
            import re as _re

            def _mark(lbl):
                n = int(_re.findall(r'\d+', nc.get_next_instruction_name())[0])
                PHASES.append((lbl, n))

            def _ln1():
                for j in range(JE):
                    nc.gpsimd.dma_start(rA[j][:], xT_d[j * 128:(j + 1) * 128, :])
                ln_phase(rA, lnT, 'cn_g', 'cn_b', flags['cn'])
                for h in range(H):
                    nc.vector.memset(KZ[h][D:128, :], 0.0)
                    nc.vector.memset(QZ[h][D:128, :], 0.0)
                for t in range(KVS):
                    nc.vector.memset(
                        V[t][:].rearrange("p (g c) -> p g c", c=VB)[:, :, D:VB],
                        1.0)

            _mark('cross')
            attention('c', lnT, ctxT, expb_c_d, rA, rB,
                      w_d['wq_c'], w_d['wk_c'], w_d['wv_c'], w_d['wo_c'],
                      JC, L,
                      pre_loop=_ln1,
                      post_qc=lambda qc: ln_phase(rB, lnT, 'sn_g', 'sn_b',
                                                  flags['sn'], only_qc=qc))
            # w1 prefetch here: gpsimd runs it after the cross-attn eb
            # stream, long before FFN, without contending with the startup DMAs
            for j in range(JE):
                nc.gpsimd.dma_start(w1s[:, j * FF:(j + 1) * FF],
                                    w_d['w1'][j * 128:(j + 1) * 128, :])
            _mark('self')
            attention('s', lnT, lnT, expb_s_d, rB, rA,
                      w_d['wq_s'], w_d['wk_s'], w_d['wv_s'], w_d['wo_s'],
                      JE, S,
                      post_qc=lambda qc: ln_phase(rA, lnT, 'fn_g', 'fn_b',
                                                  flags['fn'], only_qc=qc))
            _mark('ffn')

            # ================= FFN =================
            for qc in range(NQ):
                qs = slice(qc * QCH, (qc + 1) * QCH)
                ybig = [psum_tile(f"P{i}", 2 * QCH) for i in range(3)]
                ypt = [ybig[i // 2][:, (i % 2) * QCH:(i % 2 + 1) * QCH]
                       for i in range(JE)]

                def emit_f1(of):
                    f1 = psum_tile(("B5", "B6")[of % 2])
                    for j in range(JE):
                        nc.tensor.matmul(
                            f1[:], w1s[:, j * FF + of * 128:j * FF + of * 128 + 128],
                            lnT[j][:, qs],
                            start=(j == 0), stop=(j == JE - 1))
                    g = wst.tile([128, QCH], F16, tag="gelu", name="gelu", bufs=2)
                    nc.scalar.activation(g[:], f1[:], AF.Gelu_apprx_tanh,
                                         bias=vap('b1', of) if flags['b1'] else 0.0)
                    return g

                def load_w2(of):
                    w2t = wst.tile([128, JE * 128], F16, tag="w2g", name="w2g",
                                   bufs=3)
                    nc.sync.dma_start(w2t[:], w_d['w2'][of * 128:(of + 1) * 128, :])
                    return w2t

                gprev = emit_f1(0)
                w2prev = load_w2(0)
                for of in range(JF):
                    gnext = emit_f1(of + 1) if of + 1 < JF else None
                    w2next = load_w2(of + 1) if of + 1 < JF else None
                    for of2 in range(JE):
                        nc.tensor.matmul(ypt[of2],
                                         w2prev[:, of2 * 128:(of2 + 1) * 128],
                                         gprev[:],
                                         start=(of == 0), stop=(of == JF - 1))
                    gprev = gnext
                    w2prev = w2next
                for of2 in range(JE):
                    yo = tr.tile([128, QCH], F32, tag="yout", name="yout", bufs=2)
                    if flags['b2']:
                        nc.vector.tensor_scalar(yo[:], ypt[of2], vap('b2', of2),
                                                None, op0=OP.add)
                    else:
                        nc.vector.tensor_copy(yo[:], ypt[of2])
                    nc.sync.dma_start(yT_d[of2 * 128:(of2 + 1) * 128, qs], yo[:])

    nc.compile()
    return nc


def kernel(**inputs):
    inp = {k: np.asarray(v, dtype=np.float32) for k, v in inputs.items()}
    triv1 = lambda v: bool(np.all(v == 1.0))
    triv0 = lambda v: bool(np.all(v == 0.0))
    flags = {
        'cn': not (triv1(inp['cn_g']) and triv0(inp['cn_b'])),
        'sn': not (triv1(inp['sn_g']) and triv0(inp['sn_b'])),
        'fn': not (triv1(inp['fn_g']) and triv0(inp['fn_b'])),
        'bq_c': not triv0(inp['bq_c']), 'bk_c': not triv0(inp['bk_c']),
        'bo_c': not triv0(inp['bo_c']), 'bq_s': not triv0(inp['bq_s']),
        'bk_s': not triv0(inp['bk_s']), 'bo_s': not triv0(inp['bo_s']),
        'b1': not triv0(inp['b1']), 'b2': not triv0(inp['b2']),
    }
    assert triv0(inp['bv_c']) and triv0(inp['bv_s']), \
        "nonzero V bias not supported by this build"

    key = tuple(sorted(flags.items()))
    if key not in _BUILT:
        _BUILT[key] = _build(flags)
    nc = _BUILT[key]

    from concourse.bass_utils import run_bass_kernel_spmd

    scale = 1.0 / np.sqrt(np.float32(D))
    f16 = lambda a: np.ascontiguousarray(a.astype(np.float16))
    com = {
        'wq_c': f16(inp['wq_c'] * scale),
        'wk_c': f16(inp['wk_c']), 'wv_c': f16(inp['wv_c']),
        'wo_c': f16(inp['wo_c']),
        'wq_s': f16(inp['wq_s'] * scale),
        'wk_s': f16(inp['wk_s']), 'wv_s': f16(inp['wv_s']),
        'wo_s': f16(inp['wo_s']),
        'w1': f16(inp['w1']), 'w2': f16(inp['w2']),
        'expb_c': f16(np.exp(inp['bias_c'].transpose(0, 2, 1))),
        'expb_s': f16(np.exp(inp['bias_s'].transpose(0, 2, 1))),
    }
    chunks = []
    for nm in ['cn_g', 'cn_b', 'sn_g', 'sn_b', 'fn_g', 'fn_b']:
        chunks.append(inp[nm].reshape(-1, 128))
    chunks.append((inp['bq_c'] * scale).reshape(-1, 128))
    for nm in ['bk_c', 'bo_c']:
        chunks.append(inp[nm].reshape(-1, 128))
    chunks.append((inp['bq_s'] * scale).reshape(-1, 128))
    for nm in ['bk_s', 'bo_s', 'b1', 'b2']:
        chunks.append(inp[nm].reshape(-1, 128))
    com['vecs'] = np.ascontiguousarray(np.concatenate(chunks, 0).T)

    in_maps = []
    for b in range(B):
        m = dict(com)
        m['xT'] = f16(inp['hidden_state'][b].T)
        m['ctxT'] = f16(inp['context'][b].T)
        in_maps.append(m)

    res = run_bass_kernel_spmd(nc, in_maps, core_ids=list(range(NCORES)),
                               trace=TRACE)
    LAST['res'] = res
    y = np.stack([res.results[c]['yT'].T for c in range(B)])
    return np.ascontiguousarray(y.astype(np.float32))


# revision 37
# speedup vs baseline: 1.1238x; 1.1238x over previous
"""Trainium2 Bass kernel for nn_BasicTransformerLayer (dense transformer layer).

Strategy:
- Data-parallel over batch: B=8, one batch element per NeuronCore, no collectives.
- All activations kept TRANSPOSED [features, tokens]; host pre-transposes.
- fp16 storage for weights + activations (halves DMA, enables DVE 2x/4x fast
  modes); all matmul accumulation in fp32 PSUM; LN statistics in fp32.
- Softmax without max-subtraction: p = exp(scores) * exp(bias) with exp(bias)
  precomputed host-side (fp16). The kv-sum (softmax denominator) is folded
  into the P@V matmul: each head's V block carries 64 extra ones-columns, so
  the PV psum rows 64..127 hold the denominator replicated across partitions
  (broadcast-free normalize).
- Score tiles are paired [128, 2*512] across 2 PSUM banks so one Exp
  activation and one eb-multiply cover 2 kv tiles.
- FFN w1 is cached whole in SBUF (fp16, prefetched early on gpsimd queue);
  w2 is streamed fp16. FFN runs without DMA stalls.
"""
import sys

sys.path.insert(0, '/opt/trn_rl_repo')

import numpy as np

E, C, H, D, FF = 768, 512, 12, 64, 3072
B, S, L = 8, 1024, 256
EPS = 1e-5
NCORES = 8
QCH = 512                  # q-chunk (matmul moving free dim)
NQ = S // QCH              # 2
JE = E // 128              # 6 feature tiles
JC = C // 128              # 4 cross-feature tiles
JF = FF // 128             # 24 ffn tiles
KVS = S // 128             # 8 self kv tiles
KVC = L // 128             # 2 cross kv tiles
VB = 128                   # per-head V block in V tiles: 64 data + 64 ones

_BUILT = {}
TRACE = False
LAST = {}
PHASES = []


def _build(flags):
    import concourse.bacc as bacc
    import concourse.mybir as mybir
    import concourse.tile as tile
    from concourse.tile import add_dep_helper

    F32 = mybir.dt.float32
    F16 = mybir.dt.float16
    AF = mybir.ActivationFunctionType
    OP = mybir.AluOpType

    nc = bacc.Bacc("TRN2", target_bir_lowering=False, debug=False,
                   enable_asserts=True, num_devices=NCORES)

    def din(name, shape, dt=F16):
        return nc.dram_tensor(name, shape, dt, kind="ExternalInput").ap()

    xT_d = din("xT", [E, S])
    ctxT_d = din("ctxT", [C, L])
    w_d = {
        'wq_c': din("wq_c", [E, E]), 'wk_c': din("wk_c", [C, E]),
        'wv_c': din("wv_c", [C, E]), 'wo_c': din("wo_c", [E, E]),
        'wq_s': din("wq_s", [E, E]), 'wk_s': din("wk_s", [E, E]),
        'wv_s': din("wv_s", [E, E]), 'wo_s': din("wo_s", [E, E]),
        'w1': din("w1", [E, FF]), 'w2': din("w2", [FF, E]),
    }
    expb_c_d = din("expb_c", [H, L, S])
    expb_s_d = din("expb_s", [H, S, S])
    # vecs: [128, NV] -- per-partition 128-chunks of all small vectors
    VIDX = {}
    _off = 0
    for nm, ln in [('cn_g', JE), ('cn_b', JE), ('sn_g', JE), ('sn_b', JE),
                   ('fn_g', JE), ('fn_b', JE), ('bq_c', JE), ('bk_c', JE),
                   ('bo_c', JE), ('bq_s', JE), ('bk_s', JE), ('bo_s', JE),
                   ('b1', JF), ('b2', JE)]:
        VIDX[nm] = _off
        _off += ln
    NV = _off
    vecs_d = din("vecs", [128, NV], F32)
    yT_d = nc.dram_tensor("yT", [E, S], F32, kind="ExternalOutput").ap()

    with tile.TileContext(nc) as tc:
        with tc.tile_pool(name="const", bufs=1) as cpool, \
             tc.tile_pool(name="acts", bufs=1) as acts, \
             tc.tile_pool(name="wst", bufs=8) as wst, \
             tc.tile_pool(name="tr", bufs=2) as tr, \
             tc.tile_pool(name="pe", bufs=6) as pepool, \
             tc.tile_pool(name="eb", bufs=4) as ebpool, \
             tc.tile_pool(name="ps", bufs=1, space="PSUM") as ps:

            def T(pool, shape, dtype, tag):
                return pool.tile(shape, dtype, tag=tag, name=tag)

            ones = T(cpool, [128, 128], F16, "ones")
            epsc = T(cpool, [128, 1], F32, "epsc")
            nc.vector.memset(epsc[:], EPS)
            nc.vector.memset(ones[:], 1.0)
            vecs = T(cpool, [128, NV], F32, "vecs")
            nc.sync.dma_start(vecs[:], vecs_d[:])
            # whole w1 cached in SBUF: col j*FF + of*128 + c = w1[j*128+p, of*128+c]
            w1s = T(cpool, [128, JE * FF], F16, "w1s")

            def vap(nm, j):
                return vecs[:, VIDX[nm] + j:VIDX[nm] + j + 1]

            # persistent activation tiles (tags reused across phases)
            rA = [T(acts, [128, S], F16, f"rA{j}") for j in range(JE)]  # xT then h2T
            rB = [T(acts, [128, S], F16, f"rB{j}") for j in range(JE)]  # h1T
            lnT = [T(acts, [128, S], F16, f"ln{j}") for j in range(JE)]
            # Per-head K/Q tiles: head data in partitions 0-63, zeros in
            # 64-127, so score matmuls are uniform 128-contraction (the PE
            # runs much slower when 64-row and 128-row weight tiles alternate)
            KZ = [T(acts, [128, S], F16, f"KZ{h}") for h in range(H)]
            QZ = [T(acts, [128, QCH], F16, f"QZ{h}") for h in range(H)]
            # V tiles: per head 64 data cols + 64 ones cols
            V = [T(acts, [128, H * VB], F16, f"V{t}") for t in range(KVS)]
            AT = [T(acts, [128, QCH], F16, f"AT{j}") for j in range(JE)]
            ctxT = [T(acts, [128, L], F16, f"cx{j}") for j in range(JC)]
            # V block layout per head: [data(64) | ones(64)] -> PV psum rows
            # 0-63 hold the PV, 64-127 the denominator. KZ/QZ zero rows and
            # V ones columns are memset inside _ln1 (regions the projection
            # copies never touch, first read ~60us in) so the Vector queue
            # isn't blocked at startup.
            for j in range(JC):
                nc.sync.dma_start(ctxT[j][:], ctxT_d[j * 128:(j + 1) * 128, :])

            # PSUM layout: 8 single-bank tags B0..B7 [128,512].
            # prologue: V-chains B0-B3, wform rot B5/B6, ln s1/s2 B4/B7
            # inner loop: sc rotation B0-B4, pv B5/B6
            # FFN: y accumulators B0-B5, f1 B6/B7
            def psum_tile(tag, n=QCH):
                return ps.tile([128, n], F32, tag=tag, name=tag)

            _rot = {'i': 0}

            def rot_sm():
                t = ("B5", "B6")[_rot['i'] % 2]
                _rot['i'] += 1
                return t

            # ---------------- layer norm (transposed layout) ----------------
            def ln_phase(src, dst, gname, bname, affine, only_qc=None):
                inv = 1.0 / float(E)
                for qc in range(NQ):
                    if only_qc is not None and qc != only_qc:
                        continue
                    qs = slice(qc * QCH, (qc + 1) * QCH)
                    sqs = []
                    for j in range(JE):
                        sq = tr.tile([128, QCH], F16, tag="sq", name="sq", bufs=6)
                        nc.vector.tensor_tensor(sq[:], src[j][:, qs],
                                                src[j][:, qs], op=OP.mult)
                        sqs.append(sq)
                    s1 = psum_tile("B4")[:]
                    s2 = psum_tile("B7")[:]
                    for j in range(JE):
                        nc.tensor.matmul(s1, ones[:, 0:128], src[j][:, qs],
                                         start=(j == 0), stop=(j == JE - 1))
                    for j in range(JE):
                        nc.tensor.matmul(s2, ones[:, 0:128], sqs[j][:],
                                         start=(j == 0), stop=(j == JE - 1))
                    t1 = tr.tile([128, QCH], F32, tag="t1m", name="t1m", bufs=1)
                    nc.scalar.activation(t1[:], s1, AF.Square, scale=inv)
                    var = tr.tile([128, QCH], F32, tag="var", name="var", bufs=1)
                    nc.vector.scalar_tensor_tensor(var[:], s2, inv, t1[:],
                                                   op0=OP.mult, op1=OP.subtract)
                    # rstd = (var+eps)^-1/2 via ln+exp: both live in the same
                    # ACT table as the attention Exp, so no table swaps
                    lnv = tr.tile([128, QCH], F32, tag="lnv", name="lnv", bufs=1)
                    nc.scalar.activation(lnv[:], var[:], AF.Ln, bias=epsc[:, 0:1])
                    rstd = tr.tile([128, QCH], F16, tag="rstd", name="rstd", bufs=1)
                    nc.scalar.activation(rstd[:], lnv[:], AF.Exp, scale=-0.5)
                    m1r = tr.tile([128, QCH], F16, tag="m1r", name="m1r", bufs=1)
                    nc.vector.scalar_tensor_tensor(m1r[:], s1, inv, rstd[:],
                                                   op0=OP.mult, op1=OP.mult)
                    for j in range(JE):
                        tmp = tr.tile([128, QCH], F16, tag="lntmp", name="lntmp",
                                      bufs=1)
                        nc.vector.tensor_tensor(tmp[:], src[j][:, qs], rstd[:],
                                                op=OP.mult)
                        if affine:
                            tmp2 = tr.tile([128, QCH], F16, tag="lntmp2",
                                           name="lntmp2")
                            nc.vector.tensor_tensor(tmp2[:], tmp[:], m1r[:],
                                                    op=OP.subtract)
                            nc.vector.tensor_scalar(dst[j][:, qs], tmp2[:],
                                                    vap(gname, j), vap(bname, j),
                                                    op0=OP.mult, op1=OP.add)
                        else:
                            nc.vector.tensor_tensor(dst[j][:, qs], tmp[:], m1r[:],
                                                    op=OP.subtract)

            # ---- projection of one 128-feature block, split into 2 per-head
            # tiles (data at partitions 0-63; rows 64-127 stay zero) ----
            def proj_heads(wd, jin, src_getter, dst, of, ks, n, bflag, bname):
                pt = psum_tile(rot_sm(), n)
                wt = wst.tile([128, JE * 128], F16, tag="wg", name="wg", bufs=3)
                nc.sync.dma_start(
                    wt[:, 0:jin * 128].rearrange("p (j c) -> p j c", j=jin),
                    wd[0:jin * 128, of * 128:(of + 1) * 128]
                    .rearrange("(j p) c -> p j c", p=128))
                for j in range(jin):
                    nc.tensor.matmul(pt[:, 0:n], wt[:, j * 128:(j + 1) * 128],
                                     src_getter(j),
                                     start=(j == 0), stop=(j == jin - 1))
                src = pt
                if bflag:
                    tmpb = tr.tile([128, QCH], F32, tag="projb", name="projb",
                                   bufs=2)
                    nc.vector.tensor_scalar(tmpb[:, 0:n], pt[:, 0:n],
                                            vap(bname, of), None, op0=OP.add)
                    src = tmpb
                nc.vector.tensor_copy(dst[2 * of][0:D, ks:ks + n],
                                      src[0:D, 0:n])
                nc.scalar.copy(dst[2 * of + 1][0:D, ks:ks + n],
                               src[D:128, 0:n])

            # ---------------- attention (shared cross/self) ----------------
            def attention(prefix, lnt, kv_src, expb_d, res_in,
                          res_out, wq, wk, wv, wo, jin_kv, kv_len,
                          pre_loop=None, post_qc=None):
                nkv = kv_len // 128

                def emit_k(of):
                    for ks in range(0, kv_len, QCH):
                        n = min(QCH, kv_len - ks)
                        proj_heads(wk, jin_kv,
                                   lambda j: kv_src[j][:, ks:ks + n],
                                   KZ, of, ks, n,
                                   flags[f'bk_{prefix}'], f'bk_{prefix}')

                def emit_vgroup(os_, tg):
                    n = min(QCH, E - os_)
                    tcnt = min(4, nkv - tg)
                    vps = [psum_tile(f"B{i}", n)[:] for i in range(tcnt)]
                    for j in range(jin_kv):
                        wt = wst.tile([128, QCH], F16, tag="wv", name="wv", bufs=2)
                        nc.sync.dma_start(wt[:, 0:n],
                                          wv[j * 128:(j + 1) * 128, os_:os_ + n])
                        for i in range(tcnt):
                            nc.tensor.matmul(
                                vps[i],
                                kv_src[j][:, (tg + i) * 128:(tg + i + 1) * 128],
                                wt[:, 0:n], start=(j == 0), stop=(j == jin_kv - 1))
                    h0, nh = os_ // D, n // D
                    for i in range(tcnt):
                        dst = V[tg + i][:].rearrange(
                            "p (g c) -> p g c", c=VB)[:, h0:h0 + nh, 0:D]
                        src = vps[i].rearrange("p (g c) -> p g c", c=D)
                        nc.scalar.copy(dst, src)

                vgroups = [(os_, tg) for os_ in range(0, E, QCH)
                           for tg in range(0, nkv, 4)]
                for i in range(max(JE, len(vgroups))):
                    if i < JE:
                        emit_k(i)
                    if i < len(vgroups):
                        emit_vgroup(*vgroups[i])

                if pre_loop is not None:
                    pre_loop()

                for qc in range(NQ):
                    PHASES.append((f'{prefix}:qc{qc}',
                                   int(__import__('re').findall(
                                       r'\d+', nc.get_next_instruction_name())[0])))
                    qs = slice(qc * QCH, (qc + 1) * QCH)
                    # Q^T for this q-chunk (scale folded into wq on host)
                    for of in range(JE):
                        proj_heads(wq, JE, lambda j: lnt[j][:, qs],
                                   QZ, of, 0, QCH,
                                   flags[f'bq_{prefix}'], f'bq_{prefix}')
                    nkt = kv_len // 128
                    seq = [(h, kvt) for h in range(H) for kvt in range(nkt)]
                    SC_TAGS = ["B0", "B1", "B2", "B3", "B4"]
                    _sr = {'i': 0}
                    state = {}

                    def load_eb(h):
                        ebts = []
                        for b0 in range(0, nkt, 2):
                            ebt = ebpool.tile([128, 2 * QCH], F16, tag="eb",
                                              name="eb", bufs=6)
                            nc.gpsimd.dma_start(
                                ebt[:].rearrange("p (t c) -> p t c", t=2),
                                expb_d[h, b0 * 128:(b0 + 2) * 128, qs]
                                .rearrange("(t p) c -> p t c", p=128))
                            ebts.append(ebt)
                        state.setdefault(h, {'tiles': []})['ebts'] = ebts

                    load_eb(0)

                    _pe_chain = {'prev': None}

                    def chain(bi):
                        if _pe_chain['prev'] is not None:
                            add_dep_helper(bi.ins, _pe_chain['prev'].ins,
                                           sync=False, reason="pe-order")
                        _pe_chain['prev'] = bi

                    def s_stage(i):
                        h, kvt = seq[i]
                        st = state.setdefault(h, {'tiles': []})
                        if kvt == 0 and h + 1 < H:
                            load_eb(h + 1)
                        sc = psum_tile(SC_TAGS[_sr['i'] % 5])
                        _sr['i'] += 1
                        chain(nc.tensor.matmul(
                            sc[:], KZ[h][:, kvt * 128:(kvt + 1) * 128],
                            QZ[h][:], start=True, stop=True))
                        pe = pepool.tile([128, QCH], F16, tag="pe", name="pe",
                                         bufs=8)
                        nc.scalar.activation(pe[:], sc[:], AF.Exp)
                        nc.vector.tensor_tensor(
                            pe[:], pe[:],
                            st['ebts'][kvt // 2][:, (kvt % 2) * QCH:
                                                 (kvt % 2 + 1) * QCH],
                            op=OP.mult)
                        st['tiles'].append(pe)

                    def a_stage(i):
                        h, kvt = seq[i]
                        st = state[h]
                        th, ph = (h * D) // 128, (h * D) % 128
                        if kvt == 0:
                            st['pv'] = psum_tile(("B5", "B6")[h % 2])
                        chain(nc.tensor.matmul(
                            st['pv'][:], V[kvt][:, h * VB:(h + 1) * VB],
                            st['tiles'][kvt][:],
                            start=(kvt == 0), stop=(kvt == nkt - 1)))
                        if kvt == nkt - 1:
                            # psum rows 0-63 = PV, 64-127 = denominator
                            # (replicated). DVE cannot partition-shift, ACT
                            # can: copy denominator down via Scalar, then
                            # aligned DVE recip + normalize.
                            den = tr.tile([128, QCH], F32, tag="den", name="den",
                                          bufs=2)
                            nc.scalar.copy(den[0:D, :], st['pv'][D:2 * D, :])
                            rec = tr.tile([128, QCH], F32, tag="rec", name="rec",
                                          bufs=2)
                            nc.vector.reciprocal_approx_fast(
                                rec[0:D, :], den[0:D, :])
                            nc.vector.tensor_tensor(AT[th][ph:ph + D, :],
                                                    st['pv'][0:D, :],
                                                    rec[0:D, :], op=OP.mult)
                            del state[h]

                    LOOK = 4
                    for i in range(len(seq) + LOOK):
                        if i < len(seq):
                            s_stage(i)
                        if i >= LOOK:
                            a_stage(i - LOOK)
                    # out-projection + residual
                    for of in range(JE):
                        pt = psum_tile(rot_sm())
                        wt = wst.tile([128, JE * 128], F16, tag="wg", name="wg",
                                      bufs=3)
                        nc.sync.dma_start(
                            wt[:].rearrange("p (j c) -> p j c", j=JE),
                            wo[0:E, of * 128:(of + 1) * 128]
                            .rearrange("(j p) c -> p j c", p=128))
                        for j in range(JE):
                            nc.tensor.matmul(pt[:], wt[:, j * 128:(j + 1) * 128],
                                             AT[j][:],
                                             start=(j == 0), stop=(j == JE - 1))
                        if flags[f'bo_{prefix}']:
                            nc.vector.scalar_tensor_tensor(
                                res_out[of][:, qs], pt[:], vap(f'bo_{prefix}', of),
                                res_in[of][:, qs], op0=OP.add, op1=OP.add)
                        else:
                            nc.vector.tensor_tensor(res_out[of][:, qs], pt[:],
                                                    res_in[of][:, qs], op=OP.add)
                    if post_qc is not None:
                        post_qc(qc)

            # ================= the layer =================
            import re as _re

            def _mark(lbl):
                n = int(_re.findall(r'\d+', nc.get_next_instruction_name())[0])
                PHASES.append((lbl, n))

            def _ln1():
                for j in range(JE):
                    nc.gpsimd.dma_start(rA[j][:], xT_d[j * 128:(j + 1) * 128, :])
                ln_phase(rA, lnT, 'cn_g', 'cn_b', flags['cn'])
                for h in range(H):
                    nc.vector.memset(KZ[h][D:128, :], 0.0)
                    nc.vector.memset(QZ[h][D:128, :], 0.0)
                for t in range(KVS):
                    nc.vector.memset(
                        V[t][:].rearrange("p (g c) -> p g c", c=VB)[:, :, D:VB],
                        1.0)

            _mark('cross')
            attention('c', lnT, ctxT, expb_c_d, rA, rB,
                      w_d['wq_c'], w_d['wk_c'], w_d['wv_c'], w_d['wo_c'],
                      JC, L,
                      pre_loop=_ln1,
                      post_qc=lambda qc: ln_phase(rB, lnT, 'sn_g', 'sn_b',
                                                  flags['sn'], only_qc=qc))
            # w1 prefetch: gpsimd executes this after the cross-attn eb
            # stream, long before FFN needs it, avoiding startup contention
            for j in range(JE):
                nc.gpsimd.dma_start(w1s[:, j * FF:(j + 1) * FF],
                                    w_d['w1'][j * 128:(j + 1) * 128, :])
            _mark('self')
            attention('s', lnT, lnT, expb_s_d, rB, rA,
                      w_d['wq_s'], w_d['wk_s'], w_d['wv_s'], w_d['wo_s'],
                      JE, S,
                      post_qc=lambda qc: ln_phase(rA, lnT, 'fn_g', 'fn_b',
                                                  flags['fn'], only_qc=qc))
            _mark('ffn')

            # ================= FFN =================
            for qc in range(NQ):
                qs = slice(qc * QCH, (qc + 1) * QCH)
                ypt = [psum_tile(f"B{i}")[:] for i in range(JE)]

                def emit_f1(of):
                    f1 = psum_tile(("B6", "B7")[of % 2])
                    for j in range(JE):
                        nc.tensor.matmul(
                            f1[:], w1s[:, j * FF + of * 128:j * FF + of * 128 + 128],
                            lnT[j][:, qs],
                            start=(j == 0), stop=(j == JE - 1))
                    g = wst.tile([128, QCH], F16, tag="gelu", name="gelu", bufs=2)
                    nc.scalar.activation(g[:], f1[:], AF.Gelu_apprx_tanh,
                                         bias=vap('b1', of) if flags['b1'] else 0.0)
                    return g

                def load_w2(of):
                    w2t = wst.tile([128, JE * 128], F16, tag="w2g", name="w2g",
                                   bufs=3)
                    nc.sync.dma_start(w2t[:], w_d['w2'][of * 128:(of + 1) * 128, :])
                    return w2t

                gprev = emit_f1(0)
                w2prev = load_w2(0)
                for of in range(JF):
                    gnext = emit_f1(of + 1) if of + 1 < JF else None
                    w2next = load_w2(of + 1) if of + 1 < JF else None
                    for of2 in range(JE):
                        nc.tensor.matmul(ypt[of2],
                                         w2prev[:, of2 * 128:(of2 + 1) * 128],
                                         gprev[:],
                                         start=(of == 0), stop=(of == JF - 1))
                    gprev = gnext
                    w2prev = w2next
                for of2 in range(JE):
                    yo = tr.tile([128, QCH], F32, tag="yout", name="yout", bufs=2)
                    if flags['b2']:
                        nc.vector.tensor_scalar(yo[:], ypt[of2], vap('b2', of2),
                                                None, op0=OP.add)
                    else:
                        nc.vector.tensor_copy(yo[:], ypt[of2])
                    nc.sync.dma_start(yT_d[of2 * 128:(of2 + 1) * 128, qs], yo[:])

    nc.compile()
    return nc


def kernel(**inputs):
    inp = {k: np.asarray(v, dtype=np.float32) for k, v in inputs.items()}
    triv1 = lambda v: bool(np.all(v == 1.0))
    triv0 = lambda v: bool(np.all(v == 0.0))
    flags = {
        'cn': not (triv1(inp['cn_g']) and triv0(inp['cn_b'])),
        'sn': not (triv1(inp['sn_g']) and triv0(inp['sn_b'])),
        'fn': not (triv1(inp['fn_g']) and triv0(inp['fn_b'])),
        'bq_c': not triv0(inp['bq_c']), 'bk_c': not triv0(inp['bk_c']),
        'bo_c': not triv0(inp['bo_c']), 'bq_s': not triv0(inp['bq_s']),
        'bk_s': not triv0(inp['bk_s']), 'bo_s': not triv0(inp['bo_s']),
        'b1': not triv0(inp['b1']), 'b2': not triv0(inp['b2']),
    }
    assert triv0(inp['bv_c']) and triv0(inp['bv_s']), \
        "nonzero V bias not supported by this build"

    key = tuple(sorted(flags.items()))
    if key not in _BUILT:
        _BUILT[key] = _build(flags)
    nc = _BUILT[key]

    from concourse.bass_utils import run_bass_kernel_spmd

    scale = 1.0 / np.sqrt(np.float32(D))
    f16 = lambda a: np.ascontiguousarray(a.astype(np.float16))
    com = {
        'wq_c': f16(inp['wq_c'] * scale),
        'wk_c': f16(inp['wk_c']), 'wv_c': f16(inp['wv_c']),
        'wo_c': f16(inp['wo_c']),
        'wq_s': f16(inp['wq_s'] * scale),
        'wk_s': f16(inp['wk_s']), 'wv_s': f16(inp['wv_s']),
        'wo_s': f16(inp['wo_s']),
        'w1': f16(inp['w1']), 'w2': f16(inp['w2']),
        'expb_c': f16(np.exp(inp['bias_c'].transpose(0, 2, 1))),
        'expb_s': f16(np.exp(inp['bias_s'].transpose(0, 2, 1))),
    }
    chunks = []
    for nm in ['cn_g', 'cn_b', 'sn_g', 'sn_b', 'fn_g', 'fn_b']:
        chunks.append(inp[nm].reshape(-1, 128))
    chunks.append((inp['bq_c'] * scale).reshape(-1, 128))
    for nm in ['bk_c', 'bo_c']:
        chunks.append(inp[nm].reshape(-1, 128))
    chunks.append((inp['bq_s'] * scale).reshape(-1, 128))
    for nm in ['bk_s', 'bo_s', 'b1', 'b2']:
        chunks.append(inp[nm].reshape(-1, 128))
    com['vecs'] = np.ascontiguousarray(np.concatenate(chunks, 0).T)

    in_maps = []
    for b in range(B):
        m = dict(com)
        m['xT'] = f16(inp['hidden_state'][b].T)
        m['ctxT'] = f16(inp['context'][b].T)
        in_maps.append(m)

    res = run_bass_kernel_spmd(nc, in_maps, core_ids=list(range(NCORES)),
                               trace=TRACE)
    LAST['res'] = res
    y = np.stack([res.results[c]['yT'].T for c in range(B)])
    return np.ascontiguousarray(y.astype(np.float32))


# revision 38
# speedup vs baseline: 1.1857x; 1.0551x over previous
"""Trainium2 Bass kernel for nn_BasicTransformerLayer (dense transformer layer).

Strategy:
- Data-parallel over batch: B=8, one batch element per NeuronCore, no collectives.
- All activations kept TRANSPOSED [features, tokens]; host pre-transposes.
- fp16 storage for weights + activations (halves DMA, enables DVE 2x/4x fast
  modes); all matmul accumulation in fp32 PSUM; LN statistics in fp32.
- Softmax without max-subtraction: p = exp(scores) * exp(bias) with exp(bias)
  precomputed host-side (fp16). The kv-sum (softmax denominator) is folded
  into the P@V matmul: each head's V block carries 64 extra ones-columns, so
  the PV psum rows 64..127 hold the denominator replicated across partitions
  (broadcast-free normalize).
- Score tiles are paired [128, 2*512] across 2 PSUM banks so one Exp
  activation and one eb-multiply cover 2 kv tiles.
- FFN w1 is cached whole in SBUF (fp16, prefetched early on gpsimd queue);
  w2 is streamed fp16. FFN runs without DMA stalls.
"""
import sys

sys.path.insert(0, '/opt/trn_rl_repo')

import numpy as np

E, C, H, D, FF = 768, 512, 12, 64, 3072
B, S, L = 8, 1024, 256
EPS = 1e-5
NCORES = 8
QCH = 512                  # q-chunk (matmul moving free dim)
NQ = S // QCH              # 2
JE = E // 128              # 6 feature tiles
JC = C // 128              # 4 cross-feature tiles
JF = FF // 128             # 24 ffn tiles
KVS = S // 128             # 8 self kv tiles
KVC = L // 128             # 2 cross kv tiles
VB = 128                   # per-head V block in V tiles: 64 data + 64 ones

_BUILT = {}
TRACE = False
LAST = {}
PHASES = []


def _build(flags):
    import concourse.bacc as bacc
    import concourse.mybir as mybir
    import concourse.tile as tile
    from concourse.tile import add_dep_helper

    F32 = mybir.dt.float32
    F16 = mybir.dt.float16
    AF = mybir.ActivationFunctionType
    OP = mybir.AluOpType

    nc = bacc.Bacc("TRN2", target_bir_lowering=False, debug=False,
                   enable_asserts=True, num_devices=NCORES)

    def din(name, shape, dt=F16):
        return nc.dram_tensor(name, shape, dt, kind="ExternalInput").ap()

    xT_d = din("xT", [E, S])
    ctxT_d = din("ctxT", [C, L])
    w_d = {
        'wq_c': din("wq_c", [E, E]), 'wk_c': din("wk_c", [C, E]),
        'wv_c': din("wv_c", [C, E]), 'wo_c': din("wo_c", [E, E]),
        'wq_s': din("wq_s", [E, E]), 'wk_s': din("wk_s", [E, E]),
        'wv_s': din("wv_s", [E, E]), 'wo_s': din("wo_s", [E, E]),
        'w1': din("w1", [E, FF]), 'w2': din("w2", [FF, E]),
    }
    expb_c_d = din("expb_c", [H, L, S])
    expb_s_d = din("expb_s", [H, S, S])
    # vecs: [128, NV] -- per-partition 128-chunks of all small vectors
    VIDX = {}
    _off = 0
    for nm, ln in [('cn_g', JE), ('cn_b', JE), ('sn_g', JE), ('sn_b', JE),
                   ('fn_g', JE), ('fn_b', JE), ('bq_c', JE), ('bk_c', JE),
                   ('bo_c', JE), ('bq_s', JE), ('bk_s', JE), ('bo_s', JE),
                   ('b1', JF), ('b2', JE)]:
        VIDX[nm] = _off
        _off += ln
    NV = _off
    vecs_d = din("vecs", [128, NV], F32)
    yT_d = nc.dram_tensor("yT", [E, S], F32, kind="ExternalOutput").ap()

    with tile.TileContext(nc) as tc:
        with tc.tile_pool(name="const", bufs=1) as cpool, \
             tc.tile_pool(name="acts", bufs=1) as acts, \
             tc.tile_pool(name="wst", bufs=8) as wst, \
             tc.tile_pool(name="tr", bufs=2) as tr, \
             tc.tile_pool(name="pe", bufs=6) as pepool, \
             tc.tile_pool(name="eb", bufs=4) as ebpool, \
             tc.tile_pool(name="ps", bufs=1, space="PSUM") as ps:

            def T(pool, shape, dtype, tag):
                return pool.tile(shape, dtype, tag=tag, name=tag)

            ones = T(cpool, [128, 128], F16, "ones")
            epsc = T(cpool, [128, 1], F32, "epsc")
            nc.vector.memset(epsc[:], EPS)
            nc.vector.memset(ones[:], 1.0)
            vecs = T(cpool, [128, NV], F32, "vecs")
            nc.sync.dma_start(vecs[:], vecs_d[:])
            # whole w1 cached in SBUF: col j*FF + of*128 + c = w1[j*128+p, of*128+c]
            w1s = T(cpool, [128, JE * FF], F16, "w1s")

            def vap(nm, j):
                return vecs[:, VIDX[nm] + j:VIDX[nm] + j + 1]

            # persistent activation tiles (tags reused across phases)
            rA = [T(acts, [128, S], F16, f"rA{j}") for j in range(JE)]  # xT then h2T
            rB = [T(acts, [128, S], F16, f"rB{j}") for j in range(JE)]  # h1T
            lnT = [T(acts, [128, S], F16, f"ln{j}") for j in range(JE)]
            # Per-head K/Q tiles: head data in partitions 0-63, zeros in
            # 64-127, so score matmuls are uniform 128-contraction (the PE
            # runs much slower when 64-row and 128-row weight tiles alternate)
            KZ = [T(acts, [128, S], F16, f"KZ{h}") for h in range(H)]
            QZ = [T(acts, [128, QCH], F16, f"QZ{h}") for h in range(H)]
            # V tiles: per head 64 data cols + 64 ones cols
            V = [T(acts, [128, H * VB], F16, f"V{t}") for t in range(KVS)]
            AT = [T(acts, [128, QCH], F16, f"AT{j}") for j in range(JE)]
            ctxT = [T(acts, [128, L], F16, f"cx{j}") for j in range(JC)]
            for h in range(H):
                nc.vector.memset(KZ[h][D:128, :], 0.0)
                nc.vector.memset(QZ[h][D:128, :], 0.0)

            # V block layout per head: [data(64) | ones(64)] -> PV psum rows
            # 0-63 hold the PV, 64-127 the softmax denominator (replicated).
            # These memsets run during the DMA-bound startup when Vector is
            # otherwise idle -- relocating them later costs more than it saves.
            for t in range(KVS):
                nc.vector.memset(
                    V[t][:].rearrange("p (g c) -> p g c", c=VB)[:, :, D:VB], 1.0)
            for j in range(JC):
                nc.sync.dma_start(ctxT[j][:], ctxT_d[j * 128:(j + 1) * 128, :])

            # PSUM layout: 8 single-bank tags B0..B7 [128,512].
            # prologue: V-chains B0-B3, wform rot B5/B6, ln s1/s2 B4/B7
            # inner loop: sc rotation B0-B4, pv B5/B6
            # FFN: y accumulators B0-B5, f1 B6/B7
            def psum_tile(tag, n=QCH):
                return ps.tile([128, n], F32, tag=tag, name=tag)

            _rot = {'i': 0}

            def rot_sm():
                t = ("B5", "B6")[_rot['i'] % 2]
                _rot['i'] += 1
                return t

            # ---------------- layer norm (transposed layout) ----------------
            def ln_phase(src, dst, gname, bname, affine, only_qc=None):
                inv = 1.0 / float(E)
                for qc in range(NQ):
                    if only_qc is not None and qc != only_qc:
                        continue
                    qs = slice(qc * QCH, (qc + 1) * QCH)
                    sqs = []
                    for j in range(JE):
                        sq = tr.tile([128, QCH], F16, tag="sq", name="sq", bufs=6)
                        nc.vector.tensor_tensor(sq[:], src[j][:, qs],
                                                src[j][:, qs], op=OP.mult)
                        sqs.append(sq)
                    s1 = psum_tile("B4")[:]
                    s2 = psum_tile("B7")[:]
                    for j in range(JE):
                        nc.tensor.matmul(s1, ones[:, 0:128], src[j][:, qs],
                                         start=(j == 0), stop=(j == JE - 1))
                    for j in range(JE):
                        nc.tensor.matmul(s2, ones[:, 0:128], sqs[j][:],
                                         start=(j == 0), stop=(j == JE - 1))
                    t1 = tr.tile([128, QCH], F32, tag="t1m", name="t1m", bufs=1)
                    nc.scalar.activation(t1[:], s1, AF.Square, scale=inv)
                    var = tr.tile([128, QCH], F32, tag="var", name="var", bufs=1)
                    nc.vector.scalar_tensor_tensor(var[:], s2, inv, t1[:],
                                                   op0=OP.mult, op1=OP.subtract)
                    # rstd = (var+eps)^-1/2 via ln+exp: both live in the same
                    # ACT table as the attention Exp, so no table swaps
                    lnv = tr.tile([128, QCH], F32, tag="lnv", name="lnv", bufs=1)
                    nc.scalar.activation(lnv[:], var[:], AF.Ln, bias=epsc[:, 0:1])
                    rstd = tr.tile([128, QCH], F16, tag="rstd", name="rstd", bufs=1)
                    nc.scalar.activation(rstd[:], lnv[:], AF.Exp, scale=-0.5)
                    m1r = tr.tile([128, QCH], F16, tag="m1r", name="m1r", bufs=1)
                    nc.vector.scalar_tensor_tensor(m1r[:], s1, inv, rstd[:],
                                                   op0=OP.mult, op1=OP.mult)
                    for j in range(JE):
                        tmp = tr.tile([128, QCH], F16, tag="lntmp", name="lntmp",
                                      bufs=1)
                        nc.vector.tensor_tensor(tmp[:], src[j][:, qs], rstd[:],
                                                op=OP.mult)
                        if affine:
                            tmp2 = tr.tile([128, QCH], F16, tag="lntmp2",
                                           name="lntmp2")
                            nc.vector.tensor_tensor(tmp2[:], tmp[:], m1r[:],
                                                    op=OP.subtract)
                            nc.vector.tensor_scalar(dst[j][:, qs], tmp2[:],
                                                    vap(gname, j), vap(bname, j),
                                                    op0=OP.mult, op1=OP.add)
                        else:
                            nc.vector.tensor_tensor(dst[j][:, qs], tmp[:], m1r[:],
                                                    op=OP.subtract)

            # ---- projection of one 128-feature block, split into 2 per-head
            # tiles (data at partitions 0-63; rows 64-127 stay zero) ----
            def proj_heads(wd, jin, src_getter, dst, of, ks, n, bflag, bname):
                pt = psum_tile(rot_sm(), n)
                wt = wst.tile([128, JE * 128], F16, tag="wg", name="wg", bufs=3)
                nc.sync.dma_start(
                    wt[:, 0:jin * 128].rearrange("p (j c) -> p j c", j=jin),
                    wd[0:jin * 128, of * 128:(of + 1) * 128]
                    .rearrange("(j p) c -> p j c", p=128))
                for j in range(jin):
                    nc.tensor.matmul(pt[:, 0:n], wt[:, j * 128:(j + 1) * 128],
                                     src_getter(j),
                                     start=(j == 0), stop=(j == jin - 1))
                src = pt
                if bflag:
                    tmpb = tr.tile([128, QCH], F32, tag="projb", name="projb",
                                   bufs=2)
                    nc.vector.tensor_scalar(tmpb[:, 0:n], pt[:, 0:n],
                                            vap(bname, of), None, op0=OP.add)
                    src = tmpb
                nc.vector.tensor_copy(dst[2 * of][0:D, ks:ks + n],
                                      src[0:D, 0:n])
                nc.scalar.copy(dst[2 * of + 1][0:D, ks:ks + n],
                               src[D:128, 0:n])

            # ---------------- attention (shared cross/self) ----------------
            def attention(prefix, lnt, kv_src, expb_d, res_in,
                          res_out, wq, wk, wv, wo, jin_kv, kv_len,
                          pre_loop=None, post_qc=None):
                nkv = kv_len // 128

                def emit_k(of):
                    for ks in range(0, kv_len, QCH):
                        n = min(QCH, kv_len - ks)
                        proj_heads(wk, jin_kv,
                                   lambda j: kv_src[j][:, ks:ks + n],
                                   KZ, of, ks, n,
                                   flags[f'bk_{prefix}'], f'bk_{prefix}')

                def emit_vgroup(os_, tg):
                    n = min(QCH, E - os_)
                    tcnt = min(4, nkv - tg)
                    vps = [psum_tile(f"B{i}", n)[:] for i in range(tcnt)]
                    for j in range(jin_kv):
                        wt = wst.tile([128, QCH], F16, tag="wv", name="wv", bufs=2)
                        nc.sync.dma_start(wt[:, 0:n],
                                          wv[j * 128:(j + 1) * 128, os_:os_ + n])
                        for i in range(tcnt):
                            nc.tensor.matmul(
                                vps[i],
                                kv_src[j][:, (tg + i) * 128:(tg + i + 1) * 128],
                                wt[:, 0:n], start=(j == 0), stop=(j == jin_kv - 1))
                    h0, nh = os_ // D, n // D
                    for i in range(tcnt):
                        dst = V[tg + i][:].rearrange(
                            "p (g c) -> p g c", c=VB)[:, h0:h0 + nh, 0:D]
                        src = vps[i].rearrange("p (g c) -> p g c", c=D)
                        nc.scalar.copy(dst, src)

                vgroups = [(os_, tg) for os_ in range(0, E, QCH)
                           for tg in range(0, nkv, 4)]
                for i in range(max(JE, len(vgroups))):
                    if i < JE:
                        emit_k(i)
                    if i < len(vgroups):
                        emit_vgroup(*vgroups[i])

                if pre_loop is not None:
                    pre_loop()

                for qc in range(NQ):
                    PHASES.append((f'{prefix}:qc{qc}',
                                   int(__import__('re').findall(
                                       r'\d+', nc.get_next_instruction_name())[0])))
                    qs = slice(qc * QCH, (qc + 1) * QCH)
                    # Q^T for this q-chunk (scale folded into wq on host)
                    for of in range(JE):
                        proj_heads(wq, JE, lambda j: lnt[j][:, qs],
                                   QZ, of, 0, QCH,
                                   flags[f'bq_{prefix}'], f'bq_{prefix}')
                    nkt = kv_len // 128
                    seq = [(h, kvt) for h in range(H) for kvt in range(nkt)]
                    SC_TAGS = ["B0", "B1", "B2", "B3", "B4"]
                    _sr = {'i': 0}
                    state = {}

                    def load_eb(h):
                        ebts = []
                        for b0 in range(0, nkt, 2):
                            ebt = ebpool.tile([128, 2 * QCH], F16, tag="eb",
                                              name="eb", bufs=6)
                            nc.gpsimd.dma_start(
                                ebt[:].rearrange("p (t c) -> p t c", t=2),
                                expb_d[h, b0 * 128:(b0 + 2) * 128, qs]
                                .rearrange("(t p) c -> p t c", p=128))
                            ebts.append(ebt)
                        state.setdefault(h, {'tiles': []})['ebts'] = ebts

                    load_eb(0)

                    _pe_chain = {'prev': None}

                    def chain(bi):
                        if _pe_chain['prev'] is not None:
                            add_dep_helper(bi.ins, _pe_chain['prev'].ins,
                                           sync=False, reason="pe-order")
                        _pe_chain['prev'] = bi

                    def s_stage(i):
                        h, kvt = seq[i]
                        st = state.setdefault(h, {'tiles': []})
                        if kvt == 0 and h + 1 < H:
                            load_eb(h + 1)
                        sc = psum_tile(SC_TAGS[_sr['i'] % 5])
                        _sr['i'] += 1
                        chain(nc.tensor.matmul(
                            sc[:], KZ[h][:, kvt * 128:(kvt + 1) * 128],
                            QZ[h][:], start=True, stop=True))
                        pe = pepool.tile([128, QCH], F16, tag="pe", name="pe",
                                         bufs=8)
                        nc.scalar.activation(pe[:], sc[:], AF.Exp)
                        nc.vector.tensor_tensor(
                            pe[:], pe[:],
                            st['ebts'][kvt // 2][:, (kvt % 2) * QCH:
                                                 (kvt % 2 + 1) * QCH],
                            op=OP.mult)
                        st['tiles'].append(pe)

                    def a_stage(i):
                        h, kvt = seq[i]
                        st = state[h]
                        th, ph = (h * D) // 128, (h * D) % 128
                        if kvt == 0:
                            st['pv'] = psum_tile(("B5", "B6")[h % 2])
                        chain(nc.tensor.matmul(
                            st['pv'][:], V[kvt][:, h * VB:(h + 1) * VB],
                            st['tiles'][kvt][:],
                            start=(kvt == 0), stop=(kvt == nkt - 1)))
                        if kvt == nkt - 1:
                            # psum rows 0-63 = PV, 64-127 = denominator
                            # (replicated). DVE cannot partition-shift, ACT
                            # can: copy denominator down via Scalar, then
                            # aligned DVE recip + normalize.
                            den = tr.tile([128, QCH], F32, tag="den", name="den",
                                          bufs=2)
                            nc.scalar.copy(den[0:D, :], st['pv'][D:2 * D, :])
                            rec = tr.tile([128, QCH], F32, tag="rec", name="rec",
                                          bufs=2)
                            nc.vector.reciprocal_approx_fast(
                                rec[0:D, :], den[0:D, :])
                            nc.vector.tensor_tensor(AT[th][ph:ph + D, :],
                                                    st['pv'][0:D, :],
                                                    rec[0:D, :], op=OP.mult)
                            del state[h]

                    LOOK = 4
                    for i in range(len(seq) + LOOK):
                        if i < len(seq):
                            s_stage(i)
                        if i >= LOOK:
                            a_stage(i - LOOK)
                    # out-projection + residual
                    for of in range(JE):
                        pt = psum_tile(rot_sm())
                        wt = wst.tile([128, JE * 128], F16, tag="wg", name="wg",
                                      bufs=3)
                        nc.sync.dma_start(
                            wt[:].rearrange("p (j c) -> p j c", j=JE),
                            wo[0:E, of * 128:(of + 1) * 128]
                            .rearrange("(j p) c -> p j c", p=128))
                        for j in range(JE):
                            nc.tensor.matmul(pt[:], wt[:, j * 128:(j + 1) * 128],
                                             AT[j][:],
                                             start=(j == 0), stop=(j == JE - 1))
                        if flags[f'bo_{prefix}']:
                            nc.vector.scalar_tensor_tensor(
                                res_out[of][:, qs], pt[:], vap(f'bo_{prefix}', of),
                                res_in[of][:, qs], op0=OP.add, op1=OP.add)
                        else:
                            nc.vector.tensor_tensor(res_out[of][:, qs], pt[:],
                                                    res_in[of][:, qs], op=OP.add)
                    if post_qc is not None:
                        post_qc(qc)

            # ================= the layer =================
            import re as _re

            def _mark(lbl):
                n = int(_re.findall(r'\d+', nc.get_next_instruction_name())[0])
                PHASES.append((lbl, n))

            def _ln1():
                for j in range(JE):
                    nc.gpsimd.dma_start(rA[j][:], xT_d[j * 128:(j + 1) * 128, :])
                ln_phase(rA, lnT, 'cn_g', 'cn_b', flags['cn'])

            _mark('cross')
            attention('c', lnT, ctxT, expb_c_d, rA, rB,
                      w_d['wq_c'], w_d['wk_c'], w_d['wv_c'], w_d['wo_c'],
                      JC, L,
                      pre_loop=_ln1,
                      post_qc=lambda qc: ln_phase(rB, lnT, 'sn_g', 'sn_b',
                                                  flags['sn'], only_qc=qc))
            # w1 prefetch: gpsimd executes this after the cross-attn eb
            # stream, long before FFN needs it, avoiding startup contention
            for j in range(JE):
                nc.gpsimd.dma_start(w1s[:, j * FF:(j + 1) * FF],
                                    w_d['w1'][j * 128:(j + 1) * 128, :])
            _mark('self')
            attention('s', lnT, lnT, expb_s_d, rB, rA,
                      w_d['wq_s'], w_d['wk_s'], w_d['wv_s'], w_d['wo_s'],
                      JE, S,
                      post_qc=lambda qc: ln_phase(rA, lnT, 'fn_g', 'fn_b',
                                                  flags['fn'], only_qc=qc))
            _mark('ffn')

            # ================= FFN =================
            for qc in range(NQ):
                qs = slice(qc * QCH, (qc + 1) * QCH)
                ypt = [psum_tile(f"B{i}")[:] for i in range(JE)]

                def emit_f1(of):
                    f1 = psum_tile(("B6", "B7")[of % 2])
                    for j in range(JE):
                        nc.tensor.matmul(
                            f1[:], w1s[:, j * FF + of * 128:j * FF + of * 128 + 128],
                            lnT[j][:, qs],
                            start=(j == 0), stop=(j == JE - 1))
                    g = wst.tile([128, QCH], F16, tag="gelu", name="gelu", bufs=2)
                    nc.scalar.activation(g[:], f1[:], AF.Gelu_apprx_tanh,
                                         bias=vap('b1', of) if flags['b1'] else 0.0)
                    return g

                def load_w2(of):
                    w2t = wst.tile([128, JE * 128], F16, tag="w2g", name="w2g",
                                   bufs=3)
                    nc.sync.dma_start(w2t[:], w_d['w2'][of * 128:(of + 1) * 128, :])
                    return w2t

                gprev = emit_f1(0)
                w2prev = load_w2(0)
                for of in range(JF):
                    gnext = emit_f1(of + 1) if of + 1 < JF else None
                    w2next = load_w2(of + 1) if of + 1 < JF else None
                    for of2 in range(JE):
                        nc.tensor.matmul(ypt[of2],
                                         w2prev[:, of2 * 128:(of2 + 1) * 128],
                                         gprev[:],
                                         start=(of == 0), stop=(of == JF - 1))
                    gprev = gnext
                    w2prev = w2next
                for of2 in range(JE):
                    yo = tr.tile([128, QCH], F32, tag="yout", name="yout", bufs=2)
                    if flags['b2']:
                        nc.vector.tensor_scalar(yo[:], ypt[of2], vap('b2', of2),
                                                None, op0=OP.add)
                    else:
                        nc.vector.tensor_copy(yo[:], ypt[of2])
                    nc.sync.dma_start(yT_d[of2 * 128:(of2 + 1) * 128, qs], yo[:])

    nc.compile()
    return nc


def kernel(**inputs):
    inp = {k: np.asarray(v, dtype=np.float32) for k, v in inputs.items()}
    triv1 = lambda v: bool(np.all(v == 1.0))
    triv0 = lambda v: bool(np.all(v == 0.0))
    flags = {
        'cn': not (triv1(inp['cn_g']) and triv0(inp['cn_b'])),
        'sn': not (triv1(inp['sn_g']) and triv0(inp['sn_b'])),
        'fn': not (triv1(inp['fn_g']) and triv0(inp['fn_b'])),
        'bq_c': not triv0(inp['bq_c']), 'bk_c': not triv0(inp['bk_c']),
        'bo_c': not triv0(inp['bo_c']), 'bq_s': not triv0(inp['bq_s']),
        'bk_s': not triv0(inp['bk_s']), 'bo_s': not triv0(inp['bo_s']),
        'b1': not triv0(inp['b1']), 'b2': not triv0(inp['b2']),
    }
    assert triv0(inp['bv_c']) and triv0(inp['bv_s']), \
        "nonzero V bias not supported by this build"

    key = tuple(sorted(flags.items()))
    if key not in _BUILT:
        _BUILT[key] = _build(flags)
    nc = _BUILT[key]

    from concourse.bass_utils import run_bass_kernel_spmd

    scale = 1.0 / np.sqrt(np.float32(D))
    f16 = lambda a: np.ascontiguousarray(a.astype(np.float16))
    com = {
        'wq_c': f16(inp['wq_c'] * scale),
        'wk_c': f16(inp['wk_c']), 'wv_c': f16(inp['wv_c']),
        'wo_c': f16(inp['wo_c']),
        'wq_s': f16(inp['wq_s'] * scale),
        'wk_s': f16(inp['wk_s']), 'wv_s': f16(inp['wv_s']),
        'wo_s': f16(inp['wo_s']),
        'w1': f16(inp['w1']), 'w2': f16(inp['w2']),
        'expb_c': f16(np.exp(inp['bias_c'].transpose(0, 2, 1))),
        'expb_s': f16(np.exp(inp['bias_s'].transpose(0, 2, 1))),
    }
    chunks = []
    for nm in ['cn_g', 'cn_b', 'sn_g', 'sn_b', 'fn_g', 'fn_b']:
        chunks.append(inp[nm].reshape(-1, 128))
    chunks.append((inp['bq_c'] * scale).reshape(-1, 128))
    for nm in ['bk_c', 'bo_c']:
        chunks.append(inp[nm].reshape(-1, 128))
    chunks.append((inp['bq_s'] * scale).reshape(-1, 128))
    for nm in ['bk_s', 'bo_s', 'b1', 'b2']:
        chunks.append(inp[nm].reshape(-1, 128))
    com['vecs'] = np.ascontiguousarray(np.concatenate(chunks, 0).T)

    in_maps = []
    for b in range(B):
        m = dict(com)
        m['xT'] = f16(inp['hidden_state'][b].T)
        m['ctxT'] = f16(inp['context'][b].T)
        in_maps.append(m)

    res = run_bass_kernel_spmd(nc, in_maps, core_ids=list(range(NCORES)),
                               trace=TRACE)
    LAST['res'] = res
    y = np.stack([res.results[c]['yT'].T for c in range(B)])
    return np.ascontiguousarray(y.astype(np.float32))
